# revision 31
# baseline (speedup 1.0000x reference)
"""Matryoshka attention Trainium2 kernel: 8-core SPMD, head-parallel, bf16.

24 heads over 3 tiers -> 3 heads per core; feedback (low-rank K/V
corrections from higher tiers) folded into dense K/V projection weights
on the host. All tensors bf16 (PSUM accumulation f32); the rel-err
budget is 2e-2 and bf16 end-to-end measures ~4e-3.

Wrapped per-batch software pipeline, built so PE never drains (the HAM
clock gate re-throttles the PE array after idle):
  proj(b):   Q^T/K^T (dk on partitions, 512-token tiles) in three psum
             passes + V token-major (N=192), x^T streamed in halves.
  attn(b):   causal attention, 512-wide q chunks, scores^T layout
             (S^T = K Q^T), exp on ACT (no max subtraction; scores are
             bounded for this problem family), causal mask as a bf16
             multiply on the 4 diagonal k-tiles (4x DVE mode),
             denominator via a ones-column appended to V, reciprocal
             broadcast across partitions with gpsimd partition_broadcast
             (Pool engine; no PE broadcast matmul).
  wo:        output projection in 128-token stripes, interleaved between
             attention units and projection passes of the NEXT batch so
             PE bubbles during ACT-bound exp stretches are filled;
             partial (BT, D) bf16 outputs summed across cores on host.
During attn(b), batch b+1's x loads, tile allocation, and projection
passes are emitted between units; the last output stripes of batch b
carry into batch b+1's early units.

PSUM (8 banks): tag X [128,2,512]x3 rotates proj Q/K pairs, V pairs,
score pairs, and wo pairs (each accumulation group owns a full bank;
matmul start=True clears its whole bank); tag N [128,512]x2 rotates the
third proj group and the attention numerator.
"""

import sys

if "/opt/trn_rl_repo" not in sys.path:
    sys.path.insert(0, "/opt/trn_rl_repo")

import numpy as np
import ml_dtypes

import concourse.bass as bass
import concourse.tile as tile
from concourse import bacc, mybir
from concourse import bass_utils

F32 = mybir.dt.float32
BF = mybir.dt.bfloat16
AF = mybir.ActivationFunctionType
NPBF = ml_dtypes.bfloat16

B, T, D = 4, 1024, 2048
BT = B * T
DK = 64
NH = 3            # heads per core
NCORES = 8
IN_OFF = [0, 256, 1024, 2048]
OUT_OFF = [0, 256, 768, 1536]
NHS = [4, 8, 12]
RANK = 8
KD = D // 128     # 16 contraction chunks
QC = T // 512     # 2 q-chunks of 512 per batch


def build_nc(reps=1):
    nc = bacc.Bacc("TRN2", target_bir_lowering=False, debug=False)
    xT = nc.dram_tensor("xT", [D, BT], BF, kind="ExternalInput")
    wqk = nc.dram_tensor("wqk", [D, 384], BF, kind="ExternalInput")
    wv = nc.dram_tensor("wv", [D, 192], BF, kind="ExternalInput")
    wo = nc.dram_tensor("wo", [256, D], BF, kind="ExternalInput")
    msk = nc.dram_tensor("msk", [128, 2048], BF, kind="ExternalInput")
    cst = nc.dram_tensor("cst", [128, 64], BF, kind="ExternalInput")
    out = nc.dram_tensor("out", [BT, D], BF, kind="ExternalOutput")

    with tile.TileContext(nc) as tc:
        with tc.tile_pool(name="pers", bufs=1) as pers, \
             tc.tile_pool(name="px", bufs=2) as px, \
             tc.tile_pool(name="pqt", bufs=2) as pqt, \
             tc.tile_pool(name="pe2", bufs=2) as pe2, \
             tc.tile_pool(name="po", bufs=2) as po, \
             tc.tile_pool(name="pps", bufs=1, space="PSUM") as pps:
            wqk_sb = pers.tile([128, KD, 384], BF)
            wv_sb = pers.tile([128, KD, 192], BF)
            wo_sb = pers.tile([128, 2, D], BF)
            msk_sb = pers.tile([128, 4, 512], BF)
            ones_sb = pers.tile([1, 64], BF)

            def load_xs(b, bl):
                """Start the x^T load for 512-token tile bl of batch b.
                Two half-DMAs so the first projection matmuls can start
                after 8 of 16 contraction chunks have landed."""
                gcol = b * 1024 + bl * 512
                xs = px.tile([128, KD, 512], BF, tag=f"xs{bl}", name="xs")
                xr = xT.ap()[:, gcol:gcol + 512].rearrange(
                    "(k p) n -> p k n", p=128)
                nc.sync.dma_start(xs[:, 0:8, :], xr[:, 0:8, :])
                nc.sync.dma_start(xs[:, 8:16, :], xr[:, 8:16, :])
                return xs

            # x tile for (0,0) first so PE can start ~7us in; weights
            # chunked in consumption order behind it.
            xs_pre = [load_xs(0, 0)]
            wqk_r = wqk.ap().rearrange("(k p) n -> p k n", p=128)
            for i in range(KD // 2):
                nc.sync.dma_start(wqk_sb[:, 2*i:2*i+2, :], wqk_r[:, 2*i:2*i+2, :])
            wv_r = wv.ap().rearrange("(k p) n -> p k n", p=128)
            for i in range(KD // 4):
                nc.sync.dma_start(wv_sb[:, 4*i:4*i+4, :], wv_r[:, 4*i:4*i+4, :])
            xs_pre.append(load_xs(0, 1))
            nc.sync.dma_start(msk_sb[:], msk.ap().rearrange("p (i n) -> p i n", i=4))
            nc.sync.dma_start(ones_sb[:], cst.ap()[0:1, 0:64])
            nc.sync.dma_start(wo_sb[:], wo.ap().rearrange("(k p) n -> p k n", p=128))

            def proj_passA(xs2, t, bl=None):
                for b_ in ((0, 1) if bl is None else (bl,)):
                    col = b_ * 512
                    xs = xs2[b_]
                    pqA = pps.tile([128, 2, 512], F32, tag="X", bufs=3,
                                   name="pqA")
                    for kd in range(KD):
                        st, sp = kd == 0, kd == KD - 1
                        nc.tensor.matmul(pqA[:, 0, :], wqk_sb[:, kd, 0:128],
                                         xs[:, kd, :], start=st, stop=sp)
                        nc.tensor.matmul(pqA[:, 1, :], wqk_sb[:, kd, 128:256],
                                         xs[:, kd, :], start=st, stop=sp)
                    nc.vector.tensor_copy(t["qtA"][:, col:col + 512],
                                          pqA[:, 0, :])
                    nc.scalar.copy(t["ktA"][:, col:col + 512], pqA[:, 1, :])

            def proj_passB(xs2, t, bl=None):
                for b_ in ((0, 1) if bl is None else (bl,)):
                    col = b_ * 512
                    xs = xs2[b_]
                    pqB = pps.tile([128, 512], F32, tag="N", bufs=2,
                                   name="pqB")
                    pv0 = pps.tile([128, 2, 192], F32, tag="X", bufs=3,
                                   name="pv0", padded_shape=[128, 2, 512])
                    for kd in range(KD):
                        st, sp = kd == 0, kd == KD - 1
                        nc.tensor.matmul(pqB[:], wqk_sb[:, kd, 256:384],
                                         xs[:, kd, :], start=st, stop=sp)
                        for s in range(2):
                            nc.tensor.matmul(pv0[:, s, :],
                                             xs[:, kd, s*128:(s+1)*128],
                                             wv_sb[:, kd, :],
                                             start=st, stop=sp)
                    nc.vector.tensor_copy(t["qtB"][0:64, col:col + 512],
                                          pqB[0:64, :])
                    nc.scalar.copy(t["ktB"][0:64, col:col + 512],
                                   pqB[64:128, :])
                    nc.vector.tensor_copy(
                        t["vhat"][:, b_*4:b_*4+2, :, 0:64],
                        pv0[:, :, :].rearrange("p s (h d) -> p s h d", h=NH))

            def proj_passC(xs2, t, bl=None):
                for b_ in ((0, 1) if bl is None else (bl,)):
                    xs = xs2[b_]
                    pv1 = pps.tile([128, 2, 192], F32, tag="X", bufs=3,
                                   name="pv1", padded_shape=[128, 2, 512])
                    for kd in range(KD):
                        st, sp = kd == 0, kd == KD - 1
                        for s in range(2):
                            nc.tensor.matmul(pv1[:, s, :],
                                             xs[:, kd, (s+2)*128:(s+3)*128],
                                             wv_sb[:, kd, :],
                                             start=st, stop=sp)
                    nc.scalar.copy(
                        t["vhat"][:, b_*4+2:b_*4+4, :, 0:64],
                        pv1[:, :, :].rearrange("p s (h d) -> p s h d", h=NH))

            def head_tiles(h, qtA, ktA, qtB, ktB):
                if h == 0:
                    return qtA, 0, ktA, 0
                if h == 1:
                    return qtA, 64, ktA, 64
                return qtB, 0, ktB, 0

            def attn_scores(h, qc, t):
                nkt = 4 * qc + 4
                qt_t, qb, kt_t, kb = head_tiles(h, t["qtA"], t["ktA"],
                                                t["qtB"], t["ktB"])
                qoff = qc * 512
                es = pe2.tile([128, 8, 512], BF, tag="es", name="es")
                rhs_q = qt_t[qb:qb + 64, qoff:qoff + 512]
                for kp in range(nkt // 2):
                    psc = pps.tile([128, 2, 512], F32, tag="X", bufs=3,
                                   name="psc")
                    for j in range(2):
                        kt = 2 * kp + j
                        nc.tensor.matmul(
                            psc[:, j, :],
                            kt_t[kb:kb + 64, kt*128:(kt+1)*128],
                            rhs_q, start=True, stop=True)
                    nc.scalar.activation(es[:, 2*kp:2*kp+2, :],
                                         psc[:], AF.Exp, scale=0.125)
                nc.vector.tensor_tensor(es[:, nkt-4:nkt, :],
                                        es[:, nkt-4:nkt, :], msk_sb[:],
                                        mybir.AluOpType.mult)
                return es

            def attn_num(h, qc, es, t):
                nkt = 4 * qc + 4
                qoff = qc * 512
                pn = pps.tile([128, 512], F32, tag="N", bufs=2, name="pn")
                for kt in range(nkt):
                    nc.tensor.matmul(pn[0:65, :], t["vhat"][:, kt, h, 0:65],
                                     es[:, kt, :],
                                     start=(kt == 0), stop=(kt == nkt - 1))
                rec = pe2.tile([1, 512], BF, tag="rec", name="rec")
                with nc.allow_low_precision(
                        reason="softmax denominator reciprocal"):
                    nc.vector.reciprocal(rec[:], pn[64:65, :])
                bc = pe2.tile([64, 512], BF, tag="bc", name="bc")
                nc.gpsimd.partition_broadcast(bc[:], rec[0:1, :], channels=64)
                if h == 0:
                    dest = t["hoA"][0:64, qoff:qoff + 512]
                elif h == 1:
                    dest = t["hoA"][64:128, qoff:qoff + 512]
                else:
                    dest = t["ktB"][64:128, qoff:qoff + 512]
                nc.vector.tensor_tensor(dest, pn[0:64, :], bc[:],
                                        mybir.AluOpType.mult)

            def wo_stripe(b, mt, t):
                hoA, ktB = t["hoA"], t["ktB"]
                ms = slice(mt * 128, mt * 128 + 128)
                osb = po.tile([128, D], BF, tag="osb", name="osb")
                pos = [pps.tile([128, 2, 512], F32, tag="X", bufs=3,
                                name="pos") for _ in range(2)]
                # group by stationary operand: 4 consecutive matmuls share
                # each lhsT (one LDWEIGHTS per group on HW)
                for nt in range(4):
                    ns = slice(nt * 512, (nt + 1) * 512)
                    nc.tensor.matmul(pos[nt // 2][:, nt % 2, :], hoA[:, ms],
                                     wo_sb[:, 0, ns], start=True, stop=False)
                for nt in range(4):
                    ns = slice(nt * 512, (nt + 1) * 512)
                    nc.tensor.matmul(pos[nt // 2][:, nt % 2, :],
                                     ktB[64:128, ms], wo_sb[64:128, 1, ns],
                                     start=False, stop=True)
                for nt in range(4):
                    ns = slice(nt * 512, (nt + 1) * 512)
                    if (mt + nt) % 2 == 0:
                        nc.vector.tensor_copy(osb[:, ns], pos[nt // 2][:, nt % 2, :])
                    else:
                        nc.scalar.copy(osb[:, ns], pos[nt // 2][:, nt % 2, :])
                nc.sync.dma_start(out.ap()[b*1024 + ms.start:
                                           b*1024 + ms.stop, :], osb[:])

            def alloc_tiles():
                t = {
                    "qtA": pqt.tile([128, 1024], BF, tag="qtA", name="qtA"),
                    "ktA": pqt.tile([128, 1024], BF, tag="ktA", name="ktA"),
                    "qtB": pqt.tile([128, 1024], BF, tag="qtB", name="qtB"),
                    "ktB": pqt.tile([128, 1024], BF, tag="ktB", name="ktB"),
                    "vhat": pqt.tile([128, 8, NH, 72], BF, tag="vh",
                                     name="vhat"),
                    "hoA": pqt.tile([128, 1024], BF, tag="hoA", name="hoA"),
                }
                nc.gpsimd.memset(t["vhat"][:, :, :, 64:65], 1.0)
                return t

            def emit(xs_pre=None):
                # Wrapped software pipeline over batches: during batch b's
                # attention, batch b+1's x loads + tile alloc + first
                # projection passes are interleaved so PE never drains
                # (HAM clock gate re-throttles after idle).
                xs_cur = xs_pre if xs_pre is not None else [load_xs(0, 0),
                                                            load_xs(0, 1)]
                t = alloc_tiles()
                proj_passA(xs_cur, t)
                proj_passB(xs_cur, t)
                proj_passC(xs_cur, t)
                carry = []   # previous batch's last stripes: (b, mt, tiles)
                for b in range(B):
                    last = b + 1 >= B
                    units = [(h, qc) for qc in range(QC) for h in range(NH)]
                    prev = None
                    wo_q = []
                    for idx, (h, qc) in enumerate(units):
                        es = attn_scores(h, qc, t)
                        if prev is not None:
                            attn_num(prev[0], prev[1], prev[2], t)
                            if prev[0] == NH - 1:
                                wo_q.extend(range(4 * prev[1],
                                                  4 * prev[1] + 4))
                        if not last:
                            if idx == 0:
                                xs_nxt = [load_xs(b + 1, 0)]
                            elif idx == 1:
                                xs_nxt.append(load_xs(b + 1, 1))
                            elif idx == 3:
                                t_nxt = alloc_tiles()
                            elif idx == 4:
                                proj_passA(xs_nxt, t_nxt, bl=0)
                            elif idx == 5:
                                proj_passB(xs_nxt, t_nxt, bl=0)
                        if carry:
                            wo_stripe(*carry.pop(0))
                        elif wo_q:
                            wo_stripe(b, wo_q.pop(0), t)
                        prev = (h, qc, es)
                    attn_num(prev[0], prev[1], prev[2], t)
                    wo_q.extend(range(4 * prev[1], 4 * prev[1] + 4))
                    if last:
                        for args in carry:
                            wo_stripe(*args)
                        for mt in wo_q:
                            wo_stripe(b, mt, t)
                    else:
                        proj_passC(xs_nxt, t_nxt, bl=0)
                        wo_stripe(b, wo_q.pop(0), t)
                        proj_passA(xs_nxt, t_nxt, bl=1)
                        wo_stripe(b, wo_q.pop(0), t)
                        proj_passB(xs_nxt, t_nxt, bl=1)
                        wo_stripe(b, wo_q.pop(0), t)
                        proj_passC(xs_nxt, t_nxt, bl=1)
                        carry = [(b, mt, t) for mt in wo_q]
                        t = t_nxt

            if reps == 1:
                emit(xs_pre)
            elif reps < 0:
                for _ in range(-reps):
                    emit()
            else:
                with tc.For_i(0, reps, 1):
                    emit()
    nc.compile()
    return nc


def prep_in_maps(x, W_Q, W_K, W_V, W_O, FK0, PK0, FV0, PV0, FK1, PK1, FV1, PV1):
    x = np.asarray(x, dtype=np.float32)
    W_K_eff = np.array(W_K, dtype=np.float32, copy=True)
    W_V_eff = np.array(W_V, dtype=np.float32, copy=True)
    for tier, (FK, PK, FV, PV) in {0: (FK0, PK0, FV0, PV0),
                                   1: (FK1, PK1, FV1, PV1)}.items():
        FK = np.asarray(FK); PK = np.asarray(PK)
        FV = np.asarray(FV); PV = np.asarray(PV)
        lo = IN_OFF[tier + 1]
        for h in range(NHS[tier]):
            col = OUT_OFF[tier] + h * DK
            W_K_eff[lo:, col:col + DK] += FK[:, h * RANK:(h + 1) * RANK] @ PK[h]
            W_V_eff[lo:, col:col + DK] += FV[:, h * RANK:(h + 1) * RANK] @ PV[h]
    W_Q = np.asarray(W_Q, dtype=np.float32)
    W_O = np.asarray(W_O, dtype=np.float32)

    xT = np.ascontiguousarray(x.reshape(BT, D).T).astype(NPBF)

    # causal mask for the 4 diagonal k-tiles of each 512-wide q chunk
    k = np.arange(128)[:, None]
    q = np.arange(512)[None, :]
    msk = np.concatenate([(q >= 128 * i + k).astype(np.float32)
                          for i in range(4)], axis=1).astype(NPBF)
    cst = np.ones((128, 64), dtype=NPBF)

    in_maps = []
    for c in range(NCORES):
        lo = c * NH * DK
        hi = lo + NH * DK
        wqkc = np.concatenate([W_Q[:, lo:lo + 128], W_K_eff[:, lo:lo + 128],
                               W_Q[:, lo + 128:hi], W_K_eff[:, lo + 128:hi]],
                              axis=1).astype(NPBF)
        wvc = np.ascontiguousarray(W_V_eff[:, lo:hi]).astype(NPBF)
        woc = np.zeros((256, D), dtype=np.float32)
        woc[0:128] = W_O[lo:lo + 128]
        woc[192:256] = W_O[lo + 128:hi]
        in_maps.append({
            "xT": xT,
            "wqk": np.ascontiguousarray(wqkc),
            "wv": wvc,
            "wo": woc.astype(NPBF),
            "msk": msk,
            "cst": cst,
        })
    return in_maps


_NC_CACHE = []


def get_nc():
    if not _NC_CACHE:
        _NC_CACHE.append(build_nc())
    return _NC_CACHE[0]


def kernel(**inputs):
    nc = get_nc()
    in_maps = prep_in_maps(**inputs)
    res = bass_utils.run_bass_kernel_spmd(nc, in_maps,
                                          core_ids=list(range(NCORES)))
    acc = res.results[0]["out"].astype(np.float32)
    for c in range(1, NCORES):
        acc += res.results[c]["out"].astype(np.float32)
    return acc.reshape(B, T, D)


# revision 46
# speedup vs baseline: 1.1514x; 1.1514x over previous
"""Matryoshka attention Trainium2 kernel: 8-core SPMD, head-parallel, bf16.

24 heads over 3 tiers -> 3 heads per core; feedback (low-rank K/V
corrections from higher tiers) folded into dense K/V projection weights
on the host. All tensors bf16 (PSUM accumulation f32); the rel-err
budget is 2e-2 and bf16 end-to-end measures ~4e-3.

Wrapped per-batch software pipeline, built so PE never drains (the HAM
clock gate re-throttles the PE array after idle):
  proj(b):   Q^T/K^T (dk on partitions, 512-token tiles) in three psum
             passes + V token-major (N=192), x^T streamed in halves.
  attn(b):   causal attention, 512-wide q chunks, scores^T layout
             (S^T = K Q^T), exp on ACT (no max subtraction; scores are
             bounded for this problem family), causal mask as a bf16
             multiply on the 4 diagonal k-tiles (4x DVE mode),
             denominator via a ones-column appended to V, reciprocal
             broadcast across partitions with gpsimd partition_broadcast
             (Pool engine; no PE broadcast matmul).
  wo:        output projection in 128-token stripes, interleaved between
             attention units and projection passes of the NEXT batch so
             PE bubbles during ACT-bound exp stretches are filled;
             partial (BT, D) bf16 outputs summed across cores on host.
During attn(b), batch b+1's x loads, tile allocation, and projection
passes are emitted between units; the last output stripes of batch b
carry into batch b+1's early units.

PSUM (8 banks): tag X [128,2,512]x3 rotates proj Q/K pairs, V pairs,
score pairs, and wo pairs (each accumulation group owns a full bank;
matmul start=True clears its whole bank); tag N [128,512]x2 rotates the
third proj group and the attention numerator.
"""

import sys

if "/opt/trn_rl_repo" not in sys.path:
    sys.path.insert(0, "/opt/trn_rl_repo")

import numpy as np
import ml_dtypes

import concourse.bass as bass
import concourse.tile as tile
from concourse import bacc, mybir
from concourse import bass_utils

F32 = mybir.dt.float32
BF = mybir.dt.bfloat16
AF = mybir.ActivationFunctionType
NPBF = ml_dtypes.bfloat16

B, T, D = 4, 1024, 2048
BT = B * T
DK = 64
NH = 3            # heads per core
NCORES = 8
IN_OFF = [0, 256, 1024, 2048]
OUT_OFF = [0, 256, 768, 1536]
NHS = [4, 8, 12]
RANK = 8
KD = D // 128     # 16 contraction chunks
QC = T // 512     # 2 q-chunks of 512 per batch


def build_nc(reps=1):
    nc = bacc.Bacc("TRN2", target_bir_lowering=False, debug=False)
    xT = nc.dram_tensor("xT", [D, BT], BF, kind="ExternalInput")
    wqk = nc.dram_tensor("wqk", [D, 384], BF, kind="ExternalInput")
    wv = nc.dram_tensor("wv", [D, 192], BF, kind="ExternalInput")
    wo = nc.dram_tensor("wo", [256, D], BF, kind="ExternalInput")
    msk = nc.dram_tensor("msk", [128, 2048], BF, kind="ExternalInput")
    cst = nc.dram_tensor("cst", [128, 64], BF, kind="ExternalInput")
    out = nc.dram_tensor("out", [BT, D], BF, kind="ExternalOutput")

    with tile.TileContext(nc) as tc:
        with tc.tile_pool(name="pers", bufs=1) as pers, \
             tc.tile_pool(name="px", bufs=2) as px, \
             tc.tile_pool(name="pqt", bufs=2) as pqt, \
             tc.tile_pool(name="pe2", bufs=2) as pe2, \
             tc.tile_pool(name="po", bufs=2) as po, \
             tc.tile_pool(name="pps", bufs=1, space="PSUM") as pps:
            wqk_sb = pers.tile([128, KD, 384], BF)
            wv_sb = pers.tile([128, KD, 192], BF)
            wo_sb = pers.tile([128, 2, D], BF)
            msk_sb = pers.tile([128, 4, 512], BF)
            ones_sb = pers.tile([1, 64], BF)

            def load_xs(b, bl, split=False):
                """Start the x^T load for 512-token tile bl of batch b.
                Two half-DMAs so the first projection matmuls can start
                after 8 of 16 contraction chunks have landed; split=True
                defers the second half (startup interleaving)."""
                gcol = b * 1024 + bl * 512
                xs = px.tile([128, KD, 512], BF, tag=f"xs{bl}", name="xs")
                xr = xT.ap()[:, gcol:gcol + 512].rearrange(
                    "(k p) n -> p k n", p=128)
                nc.sync.dma_start(xs[:, 0:8, :], xr[:, 0:8, :])
                if split:
                    return xs, (lambda: nc.sync.dma_start(xs[:, 8:16, :],
                                                          xr[:, 8:16, :]))
                nc.sync.dma_start(xs[:, 8:16, :], xr[:, 8:16, :])
                return xs

            # x half-tile for (0,0) first, then the first weight chunks, so
            # the first projection matmuls start ~4.5us in; the rest of the
            # loads follow in consumption order.
            xs00, xs00_rest = load_xs(0, 0, split=True)
            xs_pre = [xs00]
            wqk_r = wqk.ap().rearrange("(k p) n -> p k n", p=128)
            for i in range(4):
                nc.sync.dma_start(wqk_sb[:, i:i+1, :], wqk_r[:, i:i+1, :])
            xs00_rest()
            for i in range(4, KD):
                nc.sync.dma_start(wqk_sb[:, i:i+1, :], wqk_r[:, i:i+1, :])
            wv_r = wv.ap().rearrange("(k p) n -> p k n", p=128)
            for i in range(KD // 4):
                nc.sync.dma_start(wv_sb[:, 4*i:4*i+4, :], wv_r[:, 4*i:4*i+4, :])
            xs_pre.append(load_xs(0, 1))
            nc.sync.dma_start(msk_sb[:], msk.ap().rearrange("p (i n) -> p i n", i=4))
            nc.sync.dma_start(ones_sb[:], cst.ap()[0:1, 0:64])
            nc.sync.dma_start(wo_sb[:], wo.ap().rearrange("(k p) n -> p k n", p=128))

            def proj_passA(xs2, t, bl=None):
                for b_ in ((0, 1) if bl is None else (bl,)):
                    col = b_ * 512
                    xs = xs2[b_]
                    pqA = pps.tile([128, 2, 512], F32, tag="X", bufs=3,
                                   name="pqA")
                    for kd in range(KD):
                        st, sp = kd == 0, kd == KD - 1
                        nc.tensor.matmul(pqA[:, 0, :], wqk_sb[:, kd, 0:128],
                                         xs[:, kd, :], start=st, stop=sp)
                        nc.tensor.matmul(pqA[:, 1, :], wqk_sb[:, kd, 128:256],
                                         xs[:, kd, :], start=st, stop=sp)
                    nc.vector.tensor_copy(t["qtA"][:, col:col + 512],
                                          pqA[:, 0, :])
                    nc.scalar.copy(t["ktA"][:, col:col + 512], pqA[:, 1, :])

            def proj_passB(xs2, t, bl=None):
                for b_ in ((0, 1) if bl is None else (bl,)):
                    col = b_ * 512
                    xs = xs2[b_]
                    pqB = pps.tile([128, 512], F32, tag="N", bufs=2,
                                   name="pqB")
                    pv0 = pps.tile([128, 2, 192], F32, tag="X", bufs=3,
                                   name="pv0", padded_shape=[128, 2, 512])
                    for kd in range(KD):
                        st, sp = kd == 0, kd == KD - 1
                        nc.tensor.matmul(pqB[:], wqk_sb[:, kd, 256:384],
                                         xs[:, kd, :], start=st, stop=sp)
                        for s in range(2):
                            nc.tensor.matmul(pv0[:, s, :],
                                             xs[:, kd, s*128:(s+1)*128],
                                             wv_sb[:, kd, :],
                                             start=st, stop=sp)
                    nc.vector.tensor_copy(t["qtB"][0:64, col:col + 512],
                                          pqB[0:64, :])
                    nc.scalar.copy(t["ktB"][0:64, col:col + 512],
                                   pqB[64:128, :])
                    nc.vector.tensor_copy(
                        t["vhat"][:, b_*4:b_*4+2, :, 0:64],
                        pv0[:, :, :].rearrange("p s (h d) -> p s h d", h=NH))

            def proj_passC(xs2, t, bl=None):
                for b_ in ((0, 1) if bl is None else (bl,)):
                    xs = xs2[b_]
                    pv1 = pps.tile([128, 2, 192], F32, tag="X", bufs=3,
                                   name="pv1", padded_shape=[128, 2, 512])
                    for kd in range(KD):
                        st, sp = kd == 0, kd == KD - 1
                        for s in range(2):
                            nc.tensor.matmul(pv1[:, s, :],
                                             xs[:, kd, (s+2)*128:(s+3)*128],
                                             wv_sb[:, kd, :],
                                             start=st, stop=sp)
                    nc.scalar.copy(
                        t["vhat"][:, b_*4+2:b_*4+4, :, 0:64],
                        pv1[:, :, :].rearrange("p s (h d) -> p s h d", h=NH))

            def head_tiles(h, qtA, ktA, qtB, ktB):
                if h == 0:
                    return qtA, 0, ktA, 0
                if h == 1:
                    return qtA, 64, ktA, 64
                return qtB, 0, ktB, 0

            def attn_scores(h, qc, t):
                nkt = 4 * qc + 4
                qt_t, qb, kt_t, kb = head_tiles(h, t["qtA"], t["ktA"],
                                                t["qtB"], t["ktB"])
                qoff = qc * 512
                es = pe2.tile([128, 8, 512], BF, tag="es", name="es")
                rhs_q = qt_t[qb:qb + 64, qoff:qoff + 512]
                for kp in range(nkt // 2):
                    psc = pps.tile([128, 2, 512], F32, tag="X", bufs=3,
                                   name="psc")
                    for j in range(2):
                        kt = 2 * kp + j
                        nc.tensor.matmul(
                            psc[:, j, :],
                            kt_t[kb:kb + 64, kt*128:(kt+1)*128],
                            rhs_q, start=True, stop=True)
                    nc.scalar.activation(es[:, 2*kp:2*kp+2, :],
                                         psc[:], AF.Exp, scale=0.125)
                    d = 2 * kp - (nkt - 4)   # diag-pair offset into msk_sb
                    if d >= 0:
                        nc.vector.tensor_tensor(
                            es[:, 2*kp:2*kp+2, :], es[:, 2*kp:2*kp+2, :],
                            msk_sb[:, d:d+2, :], mybir.AluOpType.mult)
                return es

            def attn_num(h, qc, es, t):
                nkt = 4 * qc + 4
                qoff = qc * 512
                pn = pps.tile([128, 512], F32, tag="N", bufs=2, name="pn")
                for kt in range(nkt):
                    nc.tensor.matmul(pn[0:65, :], t["vhat"][:, kt, h, 0:65],
                                     es[:, kt, :],
                                     start=(kt == 0), stop=(kt == nkt - 1))
                rec = pe2.tile([1, 512], BF, tag="rec", name="rec")
                with nc.allow_low_precision(
                        reason="softmax denominator reciprocal"):
                    nc.vector.reciprocal(rec[:], pn[64:65, :])
                bc = pe2.tile([64, 512], BF, tag="bc", name="bc")
                nc.gpsimd.partition_broadcast(bc[:], rec[0:1, :], channels=64)
                if h == 0:
                    dest = t["hoA"][0:64, qoff:qoff + 512]
                elif h == 1:
                    dest = t["hoA"][64:128, qoff:qoff + 512]
                else:
                    dest = t["ktB"][64:128, qoff:qoff + 512]
                nc.vector.tensor_tensor(dest, pn[0:64, :], bc[:],
                                        mybir.AluOpType.mult)

            def wo_stripe(b, mt, t):
                hoA, ktB = t["hoA"], t["ktB"]
                ms = slice(mt * 128, mt * 128 + 128)
                osb = po.tile([128, D], BF, tag="osb", name="osb")
                pos = [pps.tile([128, 2, 512], F32, tag="X", bufs=3,
                                name="pos") for _ in range(2)]
                # group by stationary operand: 4 consecutive matmuls share
                # each lhsT (one LDWEIGHTS per group on HW)
                for nt in range(4):
                    ns = slice(nt * 512, (nt + 1) * 512)
                    nc.tensor.matmul(pos[nt // 2][:, nt % 2, :], hoA[:, ms],
                                     wo_sb[:, 0, ns], start=True, stop=False)
                for nt in range(4):
                    ns = slice(nt * 512, (nt + 1) * 512)
                    nc.tensor.matmul(pos[nt // 2][:, nt % 2, :],
                                     ktB[64:128, ms], wo_sb[64:128, 1, ns],
                                     start=False, stop=True)
                for nt in range(4):
                    ns = slice(nt * 512, (nt + 1) * 512)
                    if (mt + nt) % 2 == 0:
                        nc.vector.tensor_copy(osb[:, ns], pos[nt // 2][:, nt % 2, :])
                    else:
                        nc.scalar.copy(osb[:, ns], pos[nt // 2][:, nt % 2, :])
                nc.sync.dma_start(out.ap()[b*1024 + ms.start:
                                           b*1024 + ms.stop, :], osb[:])

            def alloc_tiles():
                t = {
                    "qtA": pqt.tile([128, 1024], BF, tag="qtA", name="qtA"),
                    "ktA": pqt.tile([128, 1024], BF, tag="ktA", name="ktA"),
                    "qtB": pqt.tile([128, 1024], BF, tag="qtB", name="qtB"),
                    "ktB": pqt.tile([128, 1024], BF, tag="ktB", name="ktB"),
                    "vhat": pqt.tile([128, 8, NH, 72], BF, tag="vh",
                                     name="vhat"),
                    "hoA": pqt.tile([128, 1024], BF, tag="hoA", name="hoA"),
                }
                nc.gpsimd.memset(t["vhat"][:, :, :, 64:65], 1.0)
                return t

            def emit(xs_pre=None):
                # Wrapped software pipeline over batches: during batch b's
                # attention, batch b+1's x loads + tile alloc + first
                # projection passes are interleaved so PE never drains
                # (HAM clock gate re-throttles after idle).
                xs_cur = xs_pre if xs_pre is not None else [load_xs(0, 0),
                                                            load_xs(0, 1)]
                t = alloc_tiles()
                proj_passA(xs_cur, t, bl=0)
                proj_passB(xs_cur, t, bl=0)
                proj_passC(xs_cur, t, bl=0)
                carry = []   # previous batch's last stripes: (b, mt, tiles)
                for b in range(B):
                    last = b + 1 >= B
                    units = [(h, qc) for qc in range(QC) for h in range(NH)]
                    prev = None
                    wo_q = []
                    for idx, (h, qc) in enumerate(units):
                        es = attn_scores(h, qc, t)
                        if prev is not None:
                            attn_num(prev[0], prev[1], prev[2], t)
                            if prev[0] == NH - 1:
                                wo_q.extend(range(4 * prev[1],
                                                  4 * prev[1] + 4))
                        if b == 0:
                            # tile bl=1's projection rides batch 0's qc0
                            # units (they only need the first 512 tokens)
                            if idx == 0:
                                proj_passA(xs_cur, t, bl=1)
                            elif idx == 1:
                                proj_passB(xs_cur, t, bl=1)
                            elif idx == 2:
                                proj_passC(xs_cur, t, bl=1)
                        if not last:
                            if idx == 0:
                                xs_nxt = [load_xs(b + 1, 0)]
                            elif idx == 1:
                                xs_nxt.append(load_xs(b + 1, 1))
                            elif idx == 3:
                                t_nxt = alloc_tiles()
                            elif idx == 4:
                                proj_passA(xs_nxt, t_nxt, bl=0)
                            elif idx == 5:
                                proj_passB(xs_nxt, t_nxt, bl=0)
                        if carry:
                            wo_stripe(*carry.pop(0))
                        elif wo_q:
                            wo_stripe(b, wo_q.pop(0), t)
                        prev = (h, qc, es)
                    attn_num(prev[0], prev[1], prev[2], t)
                    wo_q.extend(range(4 * prev[1], 4 * prev[1] + 4))
                    if last:
                        for args in carry:
                            wo_stripe(*args)
                        for mt in wo_q:
                            wo_stripe(b, mt, t)
                    else:
                        proj_passC(xs_nxt, t_nxt, bl=0)
                        wo_stripe(b, wo_q.pop(0), t)
                        proj_passA(xs_nxt, t_nxt, bl=1)
                        wo_stripe(b, wo_q.pop(0), t)
                        proj_passB(xs_nxt, t_nxt, bl=1)
                        wo_stripe(b, wo_q.pop(0), t)
                        proj_passC(xs_nxt, t_nxt, bl=1)
                        carry = [(b, mt, t) for mt in wo_q]
                        t = t_nxt

            if reps == 1:
                emit(xs_pre)
            elif reps < 0:
                for _ in range(-reps):
                    emit()
            else:
                with tc.For_i(0, reps, 1):
                    emit()
    nc.compile()
    return nc


def prep_in_maps(x, W_Q, W_K, W_V, W_O, FK0, PK0, FV0, PV0, FK1, PK1, FV1, PV1):
    x = np.asarray(x, dtype=np.float32)
    W_K_eff = np.array(W_K, dtype=np.float32, copy=True)
    W_V_eff = np.array(W_V, dtype=np.float32, copy=True)
    for tier, (FK, PK, FV, PV) in {0: (FK0, PK0, FV0, PV0),
                                   1: (FK1, PK1, FV1, PV1)}.items():
        FK = np.asarray(FK); PK = np.asarray(PK)
        FV = np.asarray(FV); PV = np.asarray(PV)
        lo = IN_OFF[tier + 1]
        for h in range(NHS[tier]):
            col = OUT_OFF[tier] + h * DK
            W_K_eff[lo:, col:col + DK] += FK[:, h * RANK:(h + 1) * RANK] @ PK[h]
            W_V_eff[lo:, col:col + DK] += FV[:, h * RANK:(h + 1) * RANK] @ PV[h]
    W_Q = np.asarray(W_Q, dtype=np.float32)
    W_O = np.asarray(W_O, dtype=np.float32)

    xT = np.ascontiguousarray(x.reshape(BT, D).T).astype(NPBF)

    # causal mask for the 4 diagonal k-tiles of each 512-wide q chunk
    k = np.arange(128)[:, None]
    q = np.arange(512)[None, :]
    msk = np.concatenate([(q >= 128 * i + k).astype(np.float32)
                          for i in range(4)], axis=1).astype(NPBF)
    cst = np.ones((128, 64), dtype=NPBF)

    in_maps = []
    for c in range(NCORES):
        lo = c * NH * DK
        hi = lo + NH * DK
        wqkc = np.concatenate([W_Q[:, lo:lo + 128], W_K_eff[:, lo:lo + 128],
                               W_Q[:, lo + 128:hi], W_K_eff[:, lo + 128:hi]],
                              axis=1).astype(NPBF)
        wvc = np.ascontiguousarray(W_V_eff[:, lo:hi]).astype(NPBF)
        woc = np.zeros((256, D), dtype=np.float32)
        woc[0:128] = W_O[lo:lo + 128]
        woc[192:256] = W_O[lo + 128:hi]
        in_maps.append({
            "xT": xT,
            "wqk": np.ascontiguousarray(wqkc),
            "wv": wvc,
            "wo": woc.astype(NPBF),
            "msk": msk,
            "cst": cst,
        })
    return in_maps


_NC_CACHE = []


def get_nc():
    if not _NC_CACHE:
        _NC_CACHE.append(build_nc())
    return _NC_CACHE[0]


def kernel(**inputs):
    nc = get_nc()
    in_maps = prep_in_maps(**inputs)
    res = bass_utils.run_bass_kernel_spmd(nc, in_maps,
                                          core_ids=list(range(NCORES)))
    acc = res.results[0]["out"].astype(np.float32)
    for c in range(1, NCORES):
        acc += res.results[c]["out"].astype(np.float32)
    return acc.reshape(B, T, D)
